# revision 1
# baseline (speedup 1.0000x reference)
"""Trainium2 Bass kernel for nn_NetLinkEvaluate (2-layer GCN + link decoder).

Strategy (8 NeuronCores, SPMD single program, per-core data):
  - Nodes sharded by range: core c owns rows [c*12500, (c+1)*12500).
  - Dense transforms (x@W1, z1@W2, z2@Wdec-halves) computed on the owning
    core in transposed layouts (no on-device transposes needed), shards
    AllGathered into full DRAM tables (xw1, zw2).
  - Edge aggregation (both GCN layers): edges bucketed host-side by
    (dst-owning core, 128-wide dst block), padded into 128-edge tiles.
    Per tile: indirect-DMA gather of 128 source rows from the table,
    scale by edge weight (ScalarE), build one-hot S[e, j] = (dstloc_e == j)
    (VectorE iota compare), accumulate aggT += msg.T @ S in PSUM (TensorE).
    PSUM accumulation per dst block; no scatter hazards.
  - Decode out[k] = u[s_k] + v[d_k] with [u|v] = z2 @ [A|B]: computed
    gather-free. The u-half is evaluated on the core owning s_k via a
    one-hot matmul against the SBUF-resident uv block (edges sorted by s);
    the v-half likewise on the core owning d_k. Host adds the two
    permuted halves (pure O(PE) indexing).

All indices/structure are host-prepared; only values flow through engines.
"""
import math
import numpy as np

import concourse.bass as bass
import concourse.bacc as bacc
import concourse.mybir as mybir
import concourse.tile as tile
from concourse.masks import make_identity

# Problem shapes (fixed by the task)
N = 100000
E = 1000000
PE = 200000
NFEAT = 128
NHID = 64

C = 8                       # cores
BLK = 128                   # dst block width

F32 = mybir.dt.float32
I32 = mybir.dt.int32


def _dims():
    NPC = N // C                       # nodes per core
    NBLK = math.ceil(NPC / BLK)        # node blocks per core
    NPCP = NBLK * BLK                  # padded nodes per core
    NG = C * NPCP                      # padded global table rows
    return NPC, NBLK, NPCP, NG


def _table_row(n, NPC, NPCP):
    return (n // NPC) * NPCP + (n % NPC)


def _decode_side(nodes, NPC, NBLK):
    """Bucket decode edges by owning core and 128-wide window of `nodes`.

    Returns (loc [C,128,NTd] f32 window-local ids (pad 999), pos [PE] int64
    global output slot per original edge, NTd, Tw envelope list)."""
    owner = nodes // NPC
    local = nodes - owner * NPC
    win = local // BLK
    wloc = (local % BLK).astype(np.float32)

    cnt = np.zeros((C, NBLK), dtype=np.int64)
    np.add.at(cnt, (owner, win), 1)
    Tw = np.maximum(1, np.ceil(cnt.max(axis=0) / 128).astype(np.int64))
    base = np.concatenate([[0], np.cumsum(Tw)])
    NTd = int(base[-1])

    loc = np.full((C, 128, NTd), 999.0, dtype=np.float32)
    order = np.lexsort((nodes, win, owner))
    so_own, so_win, so_wloc = owner[order], win[order], wloc[order]
    grp = so_own * NBLK + so_win
    grp_starts = np.searchsorted(grp, np.arange(C * NBLK), side="left")
    rank = np.arange(len(order)) - grp_starts[grp]
    slot = base[so_win] * 128 + rank          # slot within the core's output
    loc[so_own, slot % 128, slot // 128] = so_wloc
    pos = np.empty(len(nodes), dtype=np.int64)
    pos[order] = so_own * (NTd * 128) + slot
    return loc, pos, NTd, Tw.tolist()


def _preprocess(x, edge_index, edge_weight, pos_edge_index, W1, W2, Wdec):
    """Build per-core input maps + shared structure metadata."""
    NPC, NBLK, NPCP, NG = _dims()

    src = np.asarray(edge_index[0], dtype=np.int64)
    dst = np.asarray(edge_index[1], dtype=np.int64)
    w = np.asarray(edge_weight, dtype=np.float32)

    core_of = dst // NPC
    dloc = dst - core_of * NPC
    blk = dloc // BLK
    jloc = (dloc % BLK).astype(np.float32)
    trow = _table_row(src, NPC, NPCP).astype(np.int32)

    cnt = np.zeros((C, NBLK), dtype=np.int64)
    np.add.at(cnt, (core_of, blk), 1)
    T_b = np.maximum(1, np.ceil(cnt.max(axis=0) / 128).astype(np.int64))
    tile_base = np.concatenate([[0], np.cumsum(T_b)])
    NT = int(tile_base[-1])

    eidx = np.zeros((C, 128, NT), dtype=np.int32)
    edst = np.full((C, 128, NT), 999.0, dtype=np.float32)
    ew = np.zeros((C, 128, NT), dtype=np.float32)

    order = np.lexsort((src, blk, core_of))
    so_core, so_blk = core_of[order], blk[order]
    so_trow, so_jloc, so_w = trow[order], jloc[order], w[order]
    grp = so_core * NBLK + so_blk
    grp_starts = np.searchsorted(grp, np.arange(C * NBLK), side="left")
    pos_in_grp = np.arange(len(order)) - grp_starts[grp]
    slot_tile = tile_base[so_blk] + pos_in_grp // 128
    slot_part = pos_in_grp % 128
    eidx[so_core, slot_part, slot_tile] = so_trow
    edst[so_core, slot_part, slot_tile] = so_jloc
    ew[so_core, slot_part, slot_tile] = so_w

    # decode sides
    ps = np.asarray(pos_edge_index[0], dtype=np.int64)
    pd = np.asarray(pos_edge_index[1], dtype=np.int64)
    sloc, pos_s, NTs, Tw_s = _decode_side(ps, NPC, NBLK)
    dloc2, pos_d, NTd2, Tw_d = _decode_side(pd, NPC, NBLK)

    # transposed, zero-padded x shards
    x = np.asarray(x, dtype=np.float32)
    xT = np.zeros((C, NFEAT, NPCP), dtype=np.float32)
    for c in range(C):
        xT[c, :, :NPC] = x[c * NPC:(c + 1) * NPC, :].T

    W1 = np.asarray(W1, dtype=np.float32)
    W2 = np.asarray(W2, dtype=np.float32)
    Wdec = np.asarray(Wdec, dtype=np.float32)
    AB = np.concatenate([Wdec[:, :NHID].T, Wdec[:, NHID:].T], axis=1)  # [H, 4]
    iota2 = np.tile(np.arange(128, dtype=np.float32), (128, 1))
    iotac = np.arange(128, dtype=np.float32).reshape(128, 1)

    in_maps = []
    for c in range(C):
        in_maps.append({
            "xT": xT[c],
            "W1": W1,
            "W2s": W2,
            "AB": AB.astype(np.float32),
            "iota2": iota2,
            "iotac": iotac,
            "eidx": eidx[c],
            "edst": edst[c],
            "ew": ew[c],
            "sloc": sloc[c],
            "dloc2": dloc2[c],
        })
    meta = dict(NT=NT, T_b=T_b.tolist(), NTs=NTs, Tw_s=Tw_s,
                NTd2=NTd2, Tw_d=Tw_d, pos_s=pos_s, pos_d=pos_d)
    return in_maps, meta


def _build(meta):
    """Build the SPMD Bass program (identical across cores)."""
    NPC, NBLK, NPCP, NG = _dims()
    H = NHID
    NT, T_b = meta["NT"], meta["T_b"]
    NTs, Tw_s = meta["NTs"], meta["Tw_s"]
    NTd2, Tw_d = meta["NTd2"], meta["Tw_d"]

    nc = bacc.Bacc("TRN2", target_bir_lowering=False, debug=False, num_devices=C)

    xT_t = nc.dram_tensor("xT", [NFEAT, NPCP], F32, kind="ExternalInput")
    W1_t = nc.dram_tensor("W1", [NFEAT, H], F32, kind="ExternalInput")
    W2_t = nc.dram_tensor("W2s", [H, H], F32, kind="ExternalInput")
    AB_t = nc.dram_tensor("AB", [H, 4], F32, kind="ExternalInput")
    io_t = nc.dram_tensor("iota2", [128, 128], F32, kind="ExternalInput")
    ioc_t = nc.dram_tensor("iotac", [128, 1], F32, kind="ExternalInput")
    eidx_t = nc.dram_tensor("eidx", [128, NT], I32, kind="ExternalInput")
    edst_t = nc.dram_tensor("edst", [128, NT], F32, kind="ExternalInput")
    ew_t = nc.dram_tensor("ew", [128, NT], F32, kind="ExternalInput")
    sloc_t = nc.dram_tensor("sloc", [128, NTs], F32, kind="ExternalInput")
    dloc_t = nc.dram_tensor("dloc2", [128, NTd2], F32, kind="ExternalInput")
    uo_t = nc.dram_tensor("uo", [2, NTs * 128], F32, kind="ExternalOutput")
    vo_t = nc.dram_tensor("vo", [2, NTd2 * 128], F32, kind="ExternalOutput")

    groups = [list(range(C))]

    with tile.TileContext(nc) as tc:
        with tc.tile_pool(name="dram", bufs=1, space="DRAM") as dram, \
             tc.tile_pool(name="const", bufs=1) as cst, \
             tc.tile_pool(name="zbuf", bufs=1) as zb, \
             tc.tile_pool(name="xt", bufs=3) as xtp, \
             tc.tile_pool(name="gath", bufs=6) as gp, \
             tc.tile_pool(name="msg", bufs=4) as mp, \
             tc.tile_pool(name="sel", bufs=4) as sp, \
             tc.tile_pool(name="cpo", bufs=3) as cpo, \
             tc.tile_pool(name="dcp", bufs=4) as dcp, \
             tc.tile_pool(name="psA", bufs=2, space="PSUM") as psA, \
             tc.tile_pool(name="psB", bufs=2, space="PSUM") as psB, \
             tc.tile_pool(name="psT", bufs=2, space="PSUM") as psT, \
             tc.tile_pool(name="psD", bufs=2, space="PSUM") as psD:

            xw1_sh = dram.tile([NPCP, H], F32)
            xw1_full = dram.tile([NG, H], F32)
            zw2_sh = dram.tile([NPCP, H], F32)
            zw2_full = dram.tile([NG, H], F32)

            def load_const(name, tensor, shape, dtype=F32):
                t = cst.tile(shape, dtype, name=name)
                nc.sync.dma_start(out=t[:], in_=tensor.ap()[:])
                return t

            W1s = load_const("W1s", W1_t, [NFEAT, H])
            W2s = load_const("W2s_s", W2_t, [H, H])
            ABs = load_const("ABs", AB_t, [H, 4])
            iotas = load_const("iotas", io_t, [128, 128])
            iotacs = load_const("iotacs", ioc_t, [128, 1])
            eidxs = load_const("eidxs", eidx_t, [128, NT], I32)
            edsts = load_const("edsts", edst_t, [128, NT])
            ews = load_const("ews", ew_t, [128, NT])
            slocs = load_const("slocs", sloc_t, [128, NTs])
            dlocs = load_const("dlocs", dloc_t, [128, NTd2])

            ident = cst.tile([128, 128], F32, name="ident")
            make_identity(nc, ident[:])

            z1T = zb.tile([H, NPCP], F32)
            z2T = zb.tile([H, NPCP], F32)
            uvsb = zb.tile([128, NBLK * 4], F32)

            # ---- Phase A: xw1 = x @ W1 (shard) ----
            for b in range(NBLK):
                xt = xtp.tile([NFEAT, 128], F32, tag="xt")
                nc.sync.dma_start(out=xt[:], in_=xT_t.ap()[:, b * 128:(b + 1) * 128])
                ps = psA.tile([128, H], F32, tag="psA")
                nc.tensor.matmul(ps[:], xt[:], W1s[:], start=True, stop=True)
                cp = cpo.tile([128, H], F32, tag="cpo")
                nc.vector.tensor_copy(cp[:], ps[:])
                nc.sync.dma_start(out=xw1_sh[b * 128:(b + 1) * 128, :], in_=cp[:])

            nc.gpsimd.collective_compute(
                "AllGather", mybir.AluOpType.bypass, replica_groups=groups,
                ins=[xw1_sh.opt()], outs=[xw1_full.opt()])

            # ---- Phases B/D: edge aggregation ----
            def agg_layer(table_full, zT, relu):
                for b in range(NBLK):
                    ps = psB.tile([H, 128], F32, tag="psB")
                    t0, t1 = int(sum(T_b[:b])), int(sum(T_b[:b + 1]))
                    for t in range(t0, t1):
                        g = gp.tile([128, H], F32, tag="g")
                        nc.gpsimd.indirect_dma_start(
                            out=g[:], out_offset=None,
                            in_=table_full[:],
                            in_offset=bass.IndirectOffsetOnAxis(
                                ap=eidxs[:, t:t + 1], axis=0))
                        ms = mp.tile([128, H], F32, tag="ms")
                        nc.scalar.activation(
                            ms[:], g[:], mybir.ActivationFunctionType.Copy,
                            scale=ews[:, t:t + 1])
                        Sm = sp.tile([128, 128], F32, tag="Sm")
                        nc.vector.tensor_tensor(
                            out=Sm[:], in0=iotas[:],
                            in1=edsts[:, t:t + 1].to_broadcast([128, 128]),
                            op=mybir.AluOpType.is_equal)
                        nc.tensor.matmul(ps[:], ms[:], Sm[:],
                                         start=(t == t0), stop=(t == t1 - 1))
                    if relu:
                        nc.scalar.activation(
                            zT[:, b * 128:(b + 1) * 128], ps[:],
                            mybir.ActivationFunctionType.Relu)
                    else:
                        nc.vector.tensor_copy(zT[:, b * 128:(b + 1) * 128], ps[:])

            agg_layer(xw1_full, z1T, relu=True)

            # ---- Phase C: zw2 = z1 @ W2 ----
            for b in range(NBLK):
                ps = psA.tile([128, H], F32, tag="psA")
                nc.tensor.matmul(ps[:], z1T[:, b * 128:(b + 1) * 128], W2s[:],
                                 start=True, stop=True)
                cp = cpo.tile([128, H], F32, tag="cpo")
                nc.vector.tensor_copy(cp[:], ps[:])
                nc.sync.dma_start(out=zw2_sh[b * 128:(b + 1) * 128, :], in_=cp[:])

            nc.gpsimd.collective_compute(
                "AllGather", mybir.AluOpType.bypass, replica_groups=groups,
                ins=[zw2_sh.opt()], outs=[zw2_full.opt()])

            agg_layer(zw2_full, z2T, relu=False)

            # ---- Phase E: uv = z2 @ [A|B], kept in SBUF ----
            for b in range(NBLK):
                ps = psA.tile([128, 4], F32, tag="psA", name="psE")
                nc.tensor.matmul(ps[:], z2T[:, b * 128:(b + 1) * 128], ABs[:],
                                 start=True, stop=True)
                nc.vector.tensor_copy(uvsb[:, b * 4:(b + 1) * 4], ps[:])

            # ---- Phase F: gather-free decode halves ----
            def decode_side(locs, Tw, NTd, out_t, chan0):
                t = 0
                for wnd in range(NBLK):
                    for _ in range(Tw[wnd]):
                        # row-broadcast of per-edge window-local node ids
                        pt = psT.tile([128, 128], F32, tag="psT")
                        nc.tensor.transpose(
                            out=pt[:],
                            in_=locs[:, t:t + 1].to_broadcast([128, 128]),
                            identity=ident[:])
                        lrow = sp.tile([128, 128], F32, tag="lrow")
                        nc.vector.tensor_copy(lrow[:], pt[:])
                        S2 = sp.tile([128, 128], F32, tag="S2")
                        nc.vector.tensor_tensor(
                            out=S2[:], in0=lrow[:],
                            in1=iotacs[:, 0:1].to_broadcast([128, 128]),
                            op=mybir.AluOpType.is_equal)
                        po = psD.tile([2, 128], F32, tag="psD")
                        nc.tensor.matmul(
                            po[:], uvsb[:, wnd * 4 + chan0: wnd * 4 + chan0 + 2],
                            S2[:], start=True, stop=True)
                        cp2 = dcp.tile([2, 128], F32, tag="cp2")
                        nc.vector.tensor_copy(cp2[:], po[:])
                        nc.sync.dma_start(
                            out=out_t.ap()[:, t * 128:(t + 1) * 128], in_=cp2[:])
                        t += 1
                assert t == NTd

            decode_side(slocs, Tw_s, NTs, uo_t, 0)
            decode_side(dlocs, Tw_d, NTd2, vo_t, 2)

    nc.compile()
    return nc


def _finish(results, meta):
    """Host finishing: add the two permuted decode halves."""
    upart = np.concatenate([r["uo"].T for r in results], axis=0)  # [C*NTs*128, 2]
    vpart = np.concatenate([r["vo"].T for r in results], axis=0)
    return (upart[meta["pos_s"]] + vpart[meta["pos_d"]]).astype(np.float32)


def kernel(x, edge_index, edge_weight, pos_edge_index, W1, W2, Wdec):
    from concourse import bass_utils
    in_maps, meta = _preprocess(
        x, edge_index, edge_weight, pos_edge_index, W1, W2, Wdec)
    nc = _build(meta)
    res = bass_utils.run_bass_kernel_spmd(nc, in_maps, core_ids=list(range(C)))
    return _finish(res.results, meta)



# revision 18
# speedup vs baseline: 1.7964x; 1.7964x over previous
"""Trainium2 Bass kernel for nn_NetLinkEvaluate (2-layer GCN + link decoder).

Strategy (8 NeuronCores, SPMD single program, per-core data):
  - Nodes sharded by range: core c owns rows [c*12500, (c+1)*12500).
  - Dense transforms computed on the owning shard in bf16; shards
    AllGathered into full bf16 DRAM tables (xw1, zw2). bf16 AllGather
    measured ~74us on HW (intra-chip), f32 would be ~169us.
  - Edge aggregation (both GCN layers): edges bucketed host-side by
    (dst-owning core, 128-wide dst block), padded into 128-edge tiles.
    Per tile: one indirect-DMA gather of 128 bf16 rows (multi-offset
    batching is broken in SWDGE firmware; single-offset calls pipeline at
    ~0.3us marginal), one fused DVE tensor_scalar builds the weighted
    one-hot S'[e,j] = w_e * (dst_e == j), one bf16 TensorE matmul
    accumulates aggT += g.T @ S' into a per-dst-block PSUM bank.
  - Decode: device computes uv[n] = [z2@A | z2@B] (4 per node); the
    per-edge output out[k] = uv[s_k,0:2] + uv[d_k,2:4] is pure O(PE)
    indexing done on host (same cost as the unavoidable output permute).

HW-verified constraints honored here: matmul outputs start at PSUM bank
offsets 0 (no sub-bank packing); GPSIMD never touches PSUM; no Shared
addr_space; indirect DMA uses one offset column per call.
"""
import math
import numpy as np
import ml_dtypes

import concourse.bass as bass
import concourse.bacc as bacc
import concourse.mybir as mybir
import concourse.tile as tile

# Problem shapes (fixed by the task)
N = 100000
E = 1000000
PE = 200000
NFEAT = 128
NHID = 64

C = 8                       # cores
BLK = 128                   # dst block width

F32 = mybir.dt.float32
BF16 = mybir.dt.bfloat16
I32 = mybir.dt.int32
BF = ml_dtypes.bfloat16


def _dims():
    NPC = N // C                       # nodes per core
    NBLK = math.ceil(NPC / BLK)        # node blocks per core
    NPCP = NBLK * BLK                  # padded nodes per core
    NG = C * NPCP                      # padded global table rows
    return NPC, NBLK, NPCP, NG


def _preprocess(x, edge_index, edge_weight, pos_edge_index, W1, W2, Wdec):
    """Build per-core input maps + shared structure metadata."""
    NPC, NBLK, NPCP, NG = _dims()

    src = np.asarray(edge_index[0], dtype=np.int64)
    dst = np.asarray(edge_index[1], dtype=np.int64)
    w = np.asarray(edge_weight, dtype=np.float32)

    core_of = dst // NPC
    dloc = dst - core_of * NPC
    blk = dloc // BLK
    jloc = (dloc % BLK).astype(np.float32)
    trow = ((src // NPC) * NPCP + (src % NPC)).astype(np.int32)

    cnt = np.zeros((C, NBLK), dtype=np.int64)
    np.add.at(cnt, (core_of, blk), 1)
    T_b = np.maximum(1, np.ceil(cnt.max(axis=0) / 128).astype(np.int64))
    tile_base = np.concatenate([[0], np.cumsum(T_b)])
    NT = int(tile_base[-1])

    eidx = np.zeros((C, 128, NT), dtype=np.int32)
    edst = np.full((C, 128, NT), 999.0, dtype=np.float32)
    ew = np.zeros((C, 128, NT), dtype=np.float32)

    order = np.lexsort((src, blk, core_of))
    so_core, so_blk = core_of[order], blk[order]
    so_trow, so_jloc, so_w = trow[order], jloc[order], w[order]
    grp = so_core * NBLK + so_blk
    grp_starts = np.searchsorted(grp, np.arange(C * NBLK), side="left")
    pos_in_grp = np.arange(len(order)) - grp_starts[grp]
    slot_tile = tile_base[so_blk] + pos_in_grp // 128
    slot_part = pos_in_grp % 128
    eidx[so_core, slot_part, slot_tile] = so_trow
    edst[so_core, slot_part, slot_tile] = so_jloc
    ew[so_core, slot_part, slot_tile] = so_w

    # per-core transposed x shard, bf16
    x = np.asarray(x, dtype=np.float32)
    xT = np.zeros((C, NFEAT, NPCP), dtype=BF)
    for c in range(C):
        xT[c, :, :NPC] = x[c * NPC:(c + 1) * NPC, :].T.astype(BF)

    W1 = np.asarray(W1, dtype=np.float32).astype(BF)
    W2 = np.asarray(W2, dtype=np.float32).astype(BF)
    Wdec = np.asarray(Wdec, dtype=np.float32)
    AB = np.concatenate([Wdec[:, :NHID].T, Wdec[:, NHID:].T], axis=1).astype(BF)
    iota2 = np.tile(np.arange(128, dtype=np.float32), (128, 1)).astype(BF)

    in_maps = []
    for c in range(C):
        in_maps.append({
            "xT": xT[c],
            "W1": W1,
            "W2s": W2,
            "AB": AB,
            "iota2": iota2,
            "eidx": eidx[c],
            "edst": edst[c],
            "ew": ew[c],
        })
    ps = np.asarray(pos_edge_index[0], dtype=np.int64)
    pd = np.asarray(pos_edge_index[1], dtype=np.int64)
    meta = dict(NT=NT, T_b=T_b.tolist(), ps=ps, pd=pd)
    return in_maps, meta


def _build(meta):
    """Build the SPMD Bass program (identical across cores)."""
    NPC, NBLK, NPCP, NG = _dims()
    H = NHID
    NT, T_b = meta["NT"], meta["T_b"]

    nc = bacc.Bacc("TRN2", target_bir_lowering=False, debug=False, num_devices=C)

    xT_t = nc.dram_tensor("xT", [NFEAT, NPCP], BF16, kind="ExternalInput")
    W1_t = nc.dram_tensor("W1", [NFEAT, H], BF16, kind="ExternalInput")
    W2_t = nc.dram_tensor("W2s", [H, H], BF16, kind="ExternalInput")
    AB_t = nc.dram_tensor("AB", [H, 4], BF16, kind="ExternalInput")
    io_t = nc.dram_tensor("iota2", [128, 128], BF16, kind="ExternalInput")
    eidx_t = nc.dram_tensor("eidx", [128, NT], I32, kind="ExternalInput")
    edst_t = nc.dram_tensor("edst", [128, NT], F32, kind="ExternalInput")
    ew_t = nc.dram_tensor("ew", [128, NT], F32, kind="ExternalInput")
    uv_t = nc.dram_tensor("uv", [128, NBLK * 4], F32, kind="ExternalOutput")

    groups = [list(range(C))]

    with tile.TileContext(nc) as tc:
        with tc.tile_pool(name="dram", bufs=1, space="DRAM") as dram, \
             tc.tile_pool(name="const", bufs=1) as cst, \
             tc.tile_pool(name="zbuf", bufs=1) as zb, \
             tc.tile_pool(name="xt", bufs=3) as xtp, \
             tc.tile_pool(name="gath", bufs=8) as gp, \
             tc.tile_pool(name="sel", bufs=6) as sp, \
             tc.tile_pool(name="cpo", bufs=3) as cpo, \
             tc.tile_pool(name="psA", bufs=2, space="PSUM") as psA, \
             tc.tile_pool(name="psB", bufs=3, space="PSUM") as psB:

            xw1_sh = dram.tile([NPCP, H], BF16)
            xw1_full = dram.tile([NG, H], BF16)
            zw2_sh = dram.tile([NPCP, H], BF16)
            zw2_full = dram.tile([NG, H], BF16)

            def load_const(name, tensor, shape, dtype=F32):
                t = cst.tile(shape, dtype, name=name)
                nc.sync.dma_start(out=t[:], in_=tensor.ap()[:])
                return t

            W1s = load_const("W1s", W1_t, [NFEAT, H], BF16)
            W2s = load_const("W2s_s", W2_t, [H, H], BF16)
            ABs = load_const("ABs", AB_t, [H, 4], BF16)
            iotas = load_const("iotas", io_t, [128, 128], BF16)
            eidxs = load_const("eidxs", eidx_t, [128, NT], I32)
            edsts = load_const("edsts", edst_t, [128, NT])
            ews = load_const("ews", ew_t, [128, NT])

            z1T = zb.tile([H, NPCP], BF16)
            z2T = zb.tile([H, NPCP], BF16)
            uvsb = zb.tile([128, NBLK * 4], F32)

            # ---- Phase A: xw1 shard = x @ W1, AllGather ----
            SCH = 8                     # blocks per store chunk
            for b0 in range(0, NBLK, SCH):
                n = min(SCH, NBLK - b0)
                xt = xtp.tile([128, SCH * 128], BF16, tag="xt")
                nc.sync.dma_start(
                    out=xt[:, :n * 128],
                    in_=xT_t.ap()[:, b0 * 128:(b0 + n) * 128])
                cp = cpo.tile([128, SCH * H], BF16, tag="cpo")
                for j in range(n):
                    ps = psA.tile([128, H], F32, tag="psA")
                    nc.tensor.matmul(ps[:], xt[:, j * 128:(j + 1) * 128],
                                     W1s[:], start=True, stop=True)
                    if j % 2 == 0:
                        nc.vector.tensor_copy(cp[:, j * H:(j + 1) * H], ps[:])
                    else:
                        nc.scalar.activation(
                            cp[:, j * H:(j + 1) * H], ps[:],
                            mybir.ActivationFunctionType.Copy)
                nc.sync.dma_start(
                    out=xw1_sh[b0 * 128:(b0 + n) * 128, :].rearrange(
                        "(j p) h -> p j h", p=128),
                    in_=cp[:, :n * H])

            nc.gpsimd.collective_compute(
                "AllGather", mybir.AluOpType.bypass, replica_groups=groups,
                ins=[xw1_sh.opt()], outs=[xw1_full.opt()])

            # ---- Phases B/D: edge aggregation ----
            def agg_layer(table_full, zT, relu):
                for b in range(NBLK):
                    ps = psB.tile([H, 128], F32, tag="psB")
                    t0, t1 = int(sum(T_b[:b])), int(sum(T_b[:b + 1]))
                    for t in range(t0, t1):
                        g = gp.tile([128, H], BF16, tag="g")
                        nc.gpsimd.indirect_dma_start(
                            out=g[:], out_offset=None,
                            in_=table_full[:],
                            in_offset=bass.IndirectOffsetOnAxis(
                                ap=eidxs[:, t:t + 1], axis=0))
                        Sm = sp.tile([128, 128], BF16, tag="Sm")
                        nc.vector.tensor_scalar(
                            out=Sm[:], in0=iotas[:],
                            scalar1=edsts[:, t:t + 1], scalar2=ews[:, t:t + 1],
                            op0=mybir.AluOpType.is_equal,
                            op1=mybir.AluOpType.mult)
                        nc.tensor.matmul(ps[:], g[:], Sm[:],
                                         start=(t == t0), stop=(t == t1 - 1))
                    nc.scalar.activation(
                        zT[:, b * 128:(b + 1) * 128], ps[:],
                        mybir.ActivationFunctionType.Relu if relu
                        else mybir.ActivationFunctionType.Copy)

            agg_layer(xw1_full, z1T, relu=True)

            # ---- Phase C: zw2 (shard) = z1 @ W2, then AllGather ----
            for b0 in range(0, NBLK, SCH):
                n = min(SCH, NBLK - b0)
                cp = cpo.tile([128, SCH * H], BF16, tag="cpo2")
                for j in range(n):
                    ps = psA.tile([128, H], F32, tag="psA")
                    nc.tensor.matmul(ps[:], z1T[:, (b0 + j) * 128:
                                                  (b0 + j + 1) * 128],
                                     W2s[:], start=True, stop=True)
                    if j % 2 == 0:
                        nc.vector.tensor_copy(cp[:, j * H:(j + 1) * H], ps[:])
                    else:
                        nc.scalar.activation(
                            cp[:, j * H:(j + 1) * H], ps[:],
                            mybir.ActivationFunctionType.Copy)
                nc.sync.dma_start(
                    out=zw2_sh[b0 * 128:(b0 + n) * 128, :].rearrange(
                        "(j p) h -> p j h", p=128),
                    in_=cp[:, :n * H])

            nc.gpsimd.collective_compute(
                "AllGather", mybir.AluOpType.bypass, replica_groups=groups,
                ins=[zw2_sh.opt()], outs=[zw2_full.opt()])

            agg_layer(zw2_full, z2T, relu=False)

            # ---- Phase E: uv = z2 @ [A|B] -> output ----
            for b in range(NBLK):
                ps = psA.tile([128, 4], F32, tag="psE")
                nc.tensor.matmul(ps[:], z2T[:, b * 128:(b + 1) * 128], ABs[:],
                                 start=True, stop=True)
                if b % 2 == 0:
                    nc.vector.tensor_copy(uvsb[:, b * 4:(b + 1) * 4], ps[:])
                else:
                    nc.scalar.activation(
                        uvsb[:, b * 4:(b + 1) * 4], ps[:],
                        mybir.ActivationFunctionType.Copy)
            nc.sync.dma_start(out=uv_t.ap()[:], in_=uvsb[:])

    nc.compile()
    return nc


def _finish(results, meta):
    """Host finishing: per-node uv table -> per-edge sums (pure indexing)."""
    NPC, NBLK, NPCP, NG = _dims()
    uv = np.concatenate([
        r["uv"].reshape(128, NBLK, 4).transpose(1, 0, 2).reshape(NPCP, 4)[:NPC]
        for r in results], axis=0)                      # [N, 4]
    return (uv[meta["ps"], 0:2] + uv[meta["pd"], 2:4]).astype(np.float32)


def kernel(x, edge_index, edge_weight, pos_edge_index, W1, W2, Wdec):
    from concourse import bass_utils
    in_maps, meta = _preprocess(
        x, edge_index, edge_weight, pos_edge_index, W1, W2, Wdec)
    nc = _build(meta)
    res = bass_utils.run_bass_kernel_spmd(nc, in_maps, core_ids=list(range(C)))
    return _finish(res.results, meta)
